# revision 1
# baseline (speedup 1.0000x reference)
"""Bidirectional ConvLSTM Trainium2 kernel (8-core SPMD).

Sharding: 8 sequences = 4 batches x 2 directions; core 2b = forward for
batch b, core 2b+1 = backward (host feeds time-reversed x and that
direction's weights). No cross-core traffic during the T-loop; fusion via
one pairwise AllGather of the per-step 1x1-conv partials, BatchNorm via an
8-core AllReduce of channel stats.
"""

import numpy as np
import concourse.bass as bass
import concourse.bacc as bacc
import concourse.mybir as mybir
import concourse.tile as tile
from concourse.bass_utils import run_bass_kernel_spmd

fp32 = mybir.dt.float32
fp32r = mybir.dt.float32r
i32 = mybir.dt.int32
Alu = mybir.AluOpType
Act = mybir.ActivationFunctionType

T = 16
HID = 64
S = 4096  # 64*64 spatial
EPS = 1e-5
N_CORES = 8
CORE_IDS = list(range(N_CORES))
MAGIC = 0x5F3759DF


def emit_rsqrt(nc, pool, x_ap, out_ap, iters=3):
    """out = 1/sqrt(x) via bit-trick seed + Newton, DVE only. x_ap fp32 [P,1]."""
    P = x_ap.shape[0]
    yi = pool.tile([P, 1], i32, tag=f"rsq_i{P}")
    t = pool.tile([P, 1], fp32, tag=f"rsq_t{P}")
    nc.vector.tensor_scalar(yi[:], x_ap.bitcast(i32), 1, None,
                            Alu.logical_shift_right)
    nc.vector.tensor_scalar(yi[:], yi[:], -1, MAGIC, Alu.mult, Alu.add)
    y = yi[:].bitcast(fp32)
    for i in range(iters):
        nc.vector.tensor_mul(t[:], y, y)
        nc.vector.tensor_mul(t[:], t[:], x_ap)
        nc.vector.tensor_scalar(t[:], t[:], -0.5, 1.5, Alu.mult, Alu.add)
        nc.vector.tensor_mul(out_ap if i == iters - 1 else y, y, t[:])


def build_program(nsteps=T, reps=1, chunk=2048):
    nc = bacc.Bacc("TRN2", target_bir_lowering=False, debug=False,
                   num_devices=N_CORES)

    xs = nc.dram_tensor("xs", [nsteps, 64, S], fp32, kind="ExternalInput").ap()
    wconv = nc.dram_tensor("wconv", [9, 2, 128, 128], fp32, kind="ExternalInput").ap()
    wfu = nc.dram_tensor("wfu", [128, 64], fp32, kind="ExternalInput").ap()
    gnw = nc.dram_tensor("gnw", [2, 128], fp32, kind="ExternalInput").ap()
    gnb = nc.dram_tensor("gnb", [2, 128], fp32, kind="ExternalInput").ap()
    bnw = nc.dram_tensor("bnw", [64, 1], fp32, kind="ExternalInput").ap()
    bnb = nc.dram_tensor("bnb", [64, 1], fp32, kind="ExternalInput").ap()
    ind = nc.dram_tensor("ind", [128, 2], fp32, kind="ExternalInput").ap()
    indT = nc.dram_tensor("indT", [2, 128], fp32, kind="ExternalInput").ap()
    bind = nc.dram_tensor("bind", [128, 64], fp32, kind="ExternalInput").ap()
    bindT = nc.dram_tensor("bindT", [64, 128], fp32, kind="ExternalInput").ap()
    out = nc.dram_tensor("out", [nsteps, 64, S], fp32, kind="ExternalOutput").ap()

    psend = nc.dram_tensor("psend", [nsteps, 64, S], fp32)
    pgath = nc.dram_tensor("pgath", [2, nsteps, 64, S], fp32)
    bnps = nc.dram_tensor("bnps", [64, 2], fp32)
    bnpr = nc.dram_tensor("bnpr", [64, 2], fp32, addr_space="Shared")

    with tile.TileContext(nc) as tc:
      with tc.tile_pool(name="const", bufs=1) as cp:
        # constants used by the fusion tail — must outlive the loop pools
        bind_r = cp.tile([128, 64], fp32, tag="bindr")
        nc.sync.dma_start(bind_r[:], bind)
        bindT_r = cp.tile([64, 128], fp32, tag="bindTr")
        nc.sync.dma_start(bindT_r[:], bindT)
        bnw_sb = cp.tile([64, 1], fp32, tag="bnw")
        nc.sync.dma_start(bnw_sb[:], bnw)
        bnb_sb = cp.tile([64, 1], fp32, tag="bnb")
        nc.sync.dma_start(bnb_sb[:], bnb)

        for rep in range(reps):
          with (
            tc.tile_pool(name=f"persist{rep}", bufs=1) as pp,
            tc.tile_pool(name=f"work{rep}", bufs=2) as wp,
            tc.tile_pool(name=f"pg{rep}", bufs=3, space="PSUM") as pgpool,
            tc.tile_pool(name=f"sm{rep}", bufs=2, space="PSUM") as smpool,
          ):
            # ---- one-time prologue ----
            wr_st = pp.tile([128, 18 * 128], fp32, tag="wst")
            nc.sync.dma_start(
            wr_st[:].rearrange("k (t h m) -> k t h m", t=9, h=2),
            wconv.rearrange("t h k m -> k t h m"),
        )
            wr = pp.tile([128, 18 * 128], fp32r, tag="wr")
            nc.vector.tensor_copy(wr[:], wr_st[:])

            wfu_st = pp.tile([128, 64], fp32, tag="wfust")
            nc.sync.dma_start(wfu_st[:], wfu)
            wfu_r = pp.tile([128, 64], fp32r, tag="wfur")
            nc.vector.tensor_copy(wfu_r[:], wfu_st[:])

            ind_r = pp.tile([128, 2], fp32, tag="indr")
            nc.sync.dma_start(ind_r[:], ind)
            indT_r = pp.tile([2, 128], fp32, tag="indTr")
            nc.sync.dma_start(indT_r[:], indT)

            gnw_sb = pp.tile([128, 2], fp32, tag="gnw")
            nc.sync.dma_start(gnw_sb[:], gnw.rearrange("h p -> p h"))
            gnb_sb = pp.tile([128, 2], fp32, tag="gnb")
            nc.sync.dma_start(gnb_sb[:], gnb.rearrange("h p -> p h"))

            # persistent state
            inp0 = pp.tile([128, 66, 66], fp32r, tag="inp0")
            inp1 = pp.tile([128, 66, 66], fp32r, tag="inp1")
            nc.vector.memset(inp0[:].bitcast(fp32), 0.0)
            nc.vector.memset(inp1[:].bitcast(fp32), 0.0)
            inps = [inp0, inp1]
            if_sb = pp.tile([128, S], fp32, tag="ifsb")   # i(0:64), f(64:128)
            og_sb = pp.tile([128, S], fp32, tag="ogsb")   # g(0:64), o(64:128)
            state = pp.tile([128, S], fp32, tag="state")  # c on 64:128
            scr = pp.tile([128, S], fp32, tag="scr")      # t1/tanh_c on 64:128
            nc.vector.memset(state[64:128, :], 0.0)

            # x(0) load
            xst = wp.tile([64, S], fp32, tag="xst")
            nc.sync.dma_start(xst[:], xs[0])
            nc.vector.tensor_copy(inp0[0:64, 1:65, 1:65],
                                  xst[:].rearrange("p (a b) -> p a b", a=64))

            for t in range(nsteps):
                cur = inps[t % 2]
                nxt = inps[(t + 1) % 2]
                svs, bvs = [], []
                for half in range(2):
                    raw = if_sb if half == 0 else og_sb
                    st_t = wp.tile([128, 8, 6], fp32, tag=f"st{half}")
                    for qc in range(4):
                        pg = pgpool.tile([128, 1024], fp32, tag="pg")
                        for tap in range(9):
                            dy, dx = tap // 3, tap % 3
                            lhsT = wr[:, (tap * 2 + half) * 128:(tap * 2 + half + 1) * 128]
                            for b in range(2):
                                y0 = qc * 16 + b * 8
                                nc.tensor.matmul(
                                    pg[:, b * 512:(b + 1) * 512],
                                    lhsT,
                                    cur[:, y0 + dy:y0 + dy + 8, dx:dx + 64],
                                    start=(tap == 0), stop=(tap == 8),
                                )
                        nc.scalar.copy(raw[:, qc * 1024:(qc + 1) * 1024], pg[:])
                        for b in range(2):
                            nc.vector.bn_stats(
                                st_t[:, qc * 2 + b, :],
                                raw[:, qc * 1024 + b * 512:qc * 1024 + (b + 1) * 512])

                    # group-norm stats chain for this half
                    aggr = wp.tile([128, 2], fp32, tag=f"aggr{half}")
                    nc.vector.bn_aggr(aggr[:], st_t[:])
                    s3 = wp.tile([128, 3], fp32, tag=f"s3{half}")
                    nc.vector.tensor_copy(s3[:, 0:2], aggr[:])
                    nc.vector.tensor_mul(s3[:, 2:3], aggr[:, 0:1], aggr[:, 0:1])
                    smg = smpool.tile([2, 3], fp32, tag="sm")
                    nc.tensor.matmul(smg[:], ind_r[:], s3[:], start=True, stop=True)
                    gsb = wp.tile([2, 3], fp32, tag=f"gsb{half}")
                    nc.vector.tensor_scalar_mul(gsb[:], smg[:], 1.0 / 64.0)
                    mu2 = wp.tile([2, 1], fp32, tag=f"mu2{half}")
                    nc.vector.tensor_mul(mu2[:], gsb[:, 0:1], gsb[:, 0:1])
                    varx = wp.tile([2, 1], fp32, tag=f"varx{half}")
                    nc.vector.tensor_add(varx[:], gsb[:, 1:2], gsb[:, 2:3])
                    nc.vector.scalar_tensor_tensor(varx[:], varx[:], EPS, mu2[:],
                                                   Alu.add, Alu.subtract)
                    rstd = wp.tile([2, 1], fp32, tag=f"rstd{half}")
                    emit_rsqrt(nc, wp, varx[:], rstd[:])
                    brhs = wp.tile([2, 2], fp32, tag=f"brhs{half}")
                    nc.vector.tensor_copy(brhs[:, 0:1], rstd[:])
                    nc.vector.tensor_copy(brhs[:, 1:2], gsb[:, 0:1])
                    smb = smpool.tile([128, 2], fp32, tag="sm")
                    nc.tensor.matmul(smb[:], indT_r[:], brhs[:], start=True, stop=True)
                    sv = wp.tile([128, 1], fp32, tag=f"sv{half}")
                    nc.vector.tensor_mul(sv[:], smb[:, 0:1], gnw_sb[:, half:half + 1])
                    tv = wp.tile([128, 1], fp32, tag=f"tv{half}")
                    nc.vector.tensor_mul(tv[:], smb[:, 1:2], sv[:])
                    bv = wp.tile([128, 1], fp32, tag=f"bv{half}")
                    nc.vector.tensor_sub(bv[:], gnb_sb[:, half:half + 1], tv[:])
                    svs.append(sv)
                    bvs.append(bv)

                # normalize+activate and c/h update, chunked
                for ch in range(S // chunk):
                    sl = slice(ch * chunk, (ch + 1) * chunk)
                    nc.scalar.activation(if_sb[:, sl], if_sb[:, sl], Act.Sigmoid,
                                         bias=bvs[0][:], scale=svs[0][:])
                    nc.scalar.activation(og_sb[0:64, sl], og_sb[0:64, sl], Act.Tanh,
                                         bias=bvs[1][0:64, :], scale=svs[1][0:64, :])
                    nc.scalar.activation(og_sb[64:128, sl], og_sb[64:128, sl],
                                         Act.Sigmoid,
                                         bias=bvs[1][64:128, :], scale=svs[1][64:128, :])
                    nc.vector.tensor_mul(scr[64:128, sl], if_sb[0:64, sl],
                                         og_sb[0:64, sl])
                    nc.vector.tensor_mul(state[64:128, sl], if_sb[64:128, sl],
                                         state[64:128, sl])
                    nc.vector.tensor_add(state[64:128, sl], state[64:128, sl],
                                         scr[64:128, sl])
                    nc.scalar.activation(scr[64:128, sl], state[64:128, sl], Act.Tanh)
                    r0 = ch * (chunk // 64)
                    nrows = chunk // 64
                    nc.vector.tensor_mul(
                        nxt[64:128, 1 + r0:1 + r0 + nrows, 1:65],
                        og_sb[64:128, sl].rearrange("p (a b) -> p a b", a=nrows),
                        scr[64:128, sl].rearrange("p (a b) -> p a b", a=nrows),
                    )

                if t < nsteps - 1:
                    xst = wp.tile([64, S], fp32, tag="xst")
                    nc.sync.dma_start(xst[:], xs[t + 1])
                    nc.gpsimd.tensor_copy(nxt[0:64, 1:65, 1:65],
                                          xst[:].rearrange("p (a b) -> p a b", a=64))

                # fusion partial p_t = wfu^T @ [*, h_t]
                for k in range(8):
                    pf = smpool.tile([64, 512], fp32, tag="sm")
                    nc.tensor.matmul(pf[:], wfu_r[:],
                                     nxt[:, 1 + k * 8:1 + k * 8 + 8, 1:65],
                                     start=True, stop=True)
                    psb = wp.tile([64, 512], fp32, tag="psb")
                    if k % 2 == 0:
                        nc.scalar.copy(psb[:], pf[:])
                    else:
                        nc.vector.tensor_copy(psb[:], pf[:])
                    nc.sync.dma_start(psend[t, :, k * 512:(k + 1) * 512], psb[:])

          # ---- fusion tail ----
          nc.gpsimd.collective_compute(
            "AllGather", Alu.bypass,
            replica_groups=[[0, 1], [2, 3], [4, 5], [6, 7]],
            ins=[psend[:]], outs=[pgath[:]],
          )

          nu = nsteps // 2
          with (
            tc.tile_pool(name=f"tailp{rep}", bufs=1) as tp,
            tc.tile_pool(name=f"tailw{rep}", bufs=2) as tw,
            tc.tile_pool(name=f"tsm{rep}", bufs=2, space="PSUM") as tsm,
          ):
            F = tp.tile([128, nu, S], fp32, tag="F")
            st_t = tp.tile([128, 8 * nu, 6], fp32, tag="stT")
            for u in range(nu):
                for cc in range(2):
                    cols = slice(cc * 2048, (cc + 1) * 2048)
                    tA = tw.tile([128, 2048], fp32, tag="tA")
                    tB = tw.tile([128, 2048], fp32, tag="tB")
                    nc.sync.dma_start(tA[0:64, :], pgath[0, 2 * u, :, cols])
                    nc.sync.dma_start(tA[64:128, :], pgath[0, 2 * u + 1, :, cols])
                    nc.sync.dma_start(tB[0:64, :], pgath[1, nsteps - 1 - 2 * u, :, cols])
                    nc.sync.dma_start(tB[64:128, :], pgath[1, nsteps - 2 - 2 * u, :, cols])
                    nc.vector.tensor_add(F[:, u, cols], tA[:], tB[:])
                    for q in range(4):
                        fs = slice(cc * 2048 + q * 512, cc * 2048 + (q + 1) * 512)
                        nc.vector.bn_stats(st_t[:, u * 8 + cc * 4 + q, :],
                                           F[:, u, fs])

            aggr = tw.tile([128, 2], fp32, tag="taggr")
            nc.vector.bn_aggr(aggr[:], st_t[:])
            s2 = tw.tile([128, 2], fp32, tag="ts2")
            nc.vector.tensor_copy(s2[:, 0:1], aggr[:, 0:1])
            t128 = tw.tile([128, 1], fp32, tag="t128")
            nc.vector.tensor_mul(t128[:], aggr[:, 0:1], aggr[:, 0:1])
            nc.vector.tensor_add(s2[:, 1:2], aggr[:, 1:2], t128[:])
            smg = tsm.tile([64, 2], fp32, tag="tsm")
            nc.tensor.matmul(smg[:], bind_r[:], s2[:], start=True, stop=True)
            bsb = tw.tile([64, 2], fp32, tag="bsb")
            nc.scalar.copy(bsb[:], smg[:])
            nc.sync.dma_start(bnps[:], bsb[:])
            nc.gpsimd.collective_compute(
                "AllReduce", Alu.add,
                replica_groups=[CORE_IDS],
                ins=[bnps[:]], outs=[bnpr[:]],
            )
            s16 = tw.tile([64, 2], fp32, tag="s16")
            nc.sync.dma_start(s16[:], bnpr[:])
            mE = tw.tile([64, 2], fp32, tag="mE")
            nc.vector.tensor_scalar_mul(mE[:], s16[:], 1.0 / 16.0)
            mu2 = tw.tile([64, 1], fp32, tag="tmu2")
            nc.vector.tensor_mul(mu2[:], mE[:, 0:1], mE[:, 0:1])
            varx = tw.tile([64, 1], fp32, tag="tvarx")
            nc.vector.scalar_tensor_tensor(varx[:], mE[:, 1:2], EPS, mu2[:],
                                           Alu.add, Alu.subtract)
            rstd = tw.tile([64, 1], fp32, tag="trstd")
            emit_rsqrt(nc, tw, varx[:], rstd[:])
            brhs = tw.tile([64, 2], fp32, tag="tbrhs")
            nc.vector.tensor_mul(brhs[:, 0:1], bnw_sb[:], rstd[:])
            tv = tw.tile([64, 1], fp32, tag="ttv")
            nc.vector.tensor_mul(tv[:], mE[:, 0:1], brhs[:, 0:1])
            nc.vector.tensor_sub(brhs[:, 1:2], bnb_sb[:], tv[:])
            smb = tsm.tile([128, 2], fp32, tag="tsm")
            nc.tensor.matmul(smb[:], bindT_r[:], brhs[:], start=True, stop=True)
            svec = tw.tile([128, 1], fp32, tag="tsvec")
            nc.vector.tensor_copy(svec[:], smb[:, 0:1])
            bvec = tw.tile([128, 1], fp32, tag="tbvec")
            nc.vector.tensor_copy(bvec[:], smb[:, 1:2])

            for u in range(nu):
                for cc in range(2):
                    cols = slice(cc * 2048, (cc + 1) * 2048)
                    nc.scalar.activation(F[:, u, cols], F[:, u, cols], Act.Relu,
                                         bias=bvec[:], scale=svec[:])
                nc.sync.dma_start(out[2 * u], F[0:64, u, :])
                nc.sync.dma_start(out[2 * u + 1], F[64:128, u, :])

    nc.compile()
    return nc


def make_in_maps(x, Wf, gnf_w, gnf_b, Wb, gnb_w, gnb_b, Wfu, bn_w, bn_b,
                 nsteps=T):
    B = x.shape[0]
    perm = np.concatenate([np.arange(0, 128), np.arange(192, 256),
                           np.arange(128, 192)])
    ind_m = np.zeros((128, 2), np.float32)
    ind_m[0:64, 0] = 1.0
    ind_m[64:128, 1] = 1.0
    indT_m = np.ascontiguousarray(ind_m.T)
    bind_m = np.zeros((128, 64), np.float32)
    for c in range(64):
        bind_m[c, c] = 1.0
        bind_m[c + 64, c] = 1.0
    bindT_m = np.ascontiguousarray(bind_m.T)
    Wfu2 = np.asarray(Wfu)[:, :, 0, 0]

    in_maps = []
    for core in range(N_CORES):
        b = core // 2
        fwd = core % 2 == 0
        xb = np.asarray(x)[b].reshape(-1, 64, S)[:nsteps]
        if not fwd:
            xb = xb[::-1]
        Wd = np.asarray(Wf if fwd else Wb)[perm]
        gw = np.asarray(gnf_w if fwd else gnb_w)[perm]
        gb = np.asarray(gnf_b if fwd else gnb_b)[perm]
        wconv_m = np.empty((9, 2, 128, 128), np.float32)
        for tap in range(9):
            dy, dx = tap // 3, tap % 3
            for half in range(2):
                wconv_m[tap, half] = Wd[half * 128:(half + 1) * 128, :, dy, dx].T
        wfu_m = np.zeros((128, 64), np.float32)
        wfu_m[64:128, :] = (Wfu2[:, 0:64] if fwd else Wfu2[:, 64:128]).T
        in_maps.append({
            "xs": np.ascontiguousarray(xb),
            "wconv": wconv_m,
            "wfu": wfu_m,
            "gnw": np.ascontiguousarray(gw.reshape(2, 128)),
            "gnb": np.ascontiguousarray(gb.reshape(2, 128)),
            "bnw": np.asarray(bn_w, np.float32).reshape(64, 1).copy(),
            "bnb": np.asarray(bn_b, np.float32).reshape(64, 1).copy(),
            "ind": ind_m,
            "indT": indT_m,
            "bind": bind_m,
            "bindT": bindT_m,
        })
    return in_maps


_cached_nc = None


def kernel(x, Wf, gnf_w, gnf_b, Wb, gnb_w, gnb_b, Wfu, bn_w, bn_b):
    global _cached_nc
    if _cached_nc is None:
        _cached_nc = build_program(T)
    nc = _cached_nc
    in_maps = make_in_maps(x, Wf, gnf_w, gnf_b, Wb, gnb_w, gnb_b, Wfu,
                           bn_w, bn_b)
    res = run_bass_kernel_spmd(nc, in_maps, CORE_IDS)
    outs = [res.results[2 * b]["out"].reshape(T, HID, 64, 64)
            for b in range(4)]
    return np.ascontiguousarray(np.stack(outs).astype(np.float32))



# revision 16
# speedup vs baseline: 1.1292x; 1.1292x over previous
"""Bidirectional ConvLSTM Trainium2 kernel (8-core SPMD), v2.

Sharding: 8 sequences = 4 batches x 2 directions; core 2b = forward for
batch b, core 2b+1 = backward (host feeds time-reversed x and that
direction's weights). bf16 datapath (conv matmuls, gates, c/h state).

Fusion/BN tail is time-split: each core finalizes its LOCAL steps
j = T/2..T-1 (true t = j on fwd cores, 15-j on bwd cores). Each core's
EARLY-half fusion partials (local t < T/2) are AllGather'd pairwise
mid-loop (hidden under compute); the tail combines local psend[j] with
the peer's pgath[T-1-j] via PE matmuls using host-provided selector
weights (wsel) that encode core parity. BatchNorm stats via an 8-core
AllReduce; host reassembles the time axis from both cores of each pair.
"""

import os

import numpy as np
import concourse.bass as bass
import concourse.bacc as bacc
import concourse.mybir as mybir
import concourse.tile as tile
from concourse.bass_utils import run_bass_kernel_spmd

fp32 = mybir.dt.float32
bf16 = mybir.dt.bfloat16
fp8 = mybir.dt.float8e4
PM = mybir.MatmulPerfMode
MM_FP8 = os.environ.get("MM_FP8", "0") == "1"  # fp8e4m3 DoubleRow conv
W8SCALE = 64.0     # fp8 conv-weight prescale; cancels exactly in GroupNorm
PITCH = 80         # input-tile row pitch (16B-aligned for the DR pair stride)
ROWS = 67          # 66 + 1 guard row for the dy+1 pair read of the last taps
i32 = mybir.dt.int32
Alu = mybir.AluOpType
Act = mybir.ActivationFunctionType

T = 16
HID = 64
S = 4096  # 64*64 spatial
EPS = 1e-5
N_CORES = 8
CORE_IDS = list(range(N_CORES))
MAGIC = 0x5F3759DF
CHUNK = 1024  # gate-phase chunk = 16 spatial rows
NCH = S // CHUNK


def emit_rsqrt(nc, pool, x_ap, out_ap, iters=2):
    """out = 1/sqrt(x) via bit-trick seed + Newton, DVE only. x_ap fp32 [P,1]."""
    P = x_ap.shape[0]
    yi = pool.tile([P, 1], i32, tag=f"rsq_i{P}")
    t = pool.tile([P, 1], fp32, tag=f"rsq_t{P}")
    nc.vector.tensor_scalar(yi[:], x_ap.bitcast(i32), 1, None,
                            Alu.logical_shift_right)
    nc.vector.tensor_scalar(yi[:], yi[:], -1, MAGIC, Alu.mult, Alu.add)
    y = yi[:].bitcast(fp32)
    for i in range(iters):
        nc.vector.tensor_mul(t[:], y, y)
        nc.vector.tensor_mul(t[:], t[:], x_ap)
        nc.vector.tensor_scalar(t[:], t[:], -0.5, 1.5, Alu.mult, Alu.add)
        nc.vector.tensor_mul(out_ap if i == iters - 1 else y, y, t[:])


def build_program(nsteps=T, reps=1, with_tail=True):
    assert nsteps % 4 == 0
    nlate = nsteps // 2          # steps this core finalizes (local j >= nlate)
    nu = nlate // 2              # tail iterations (2 steps packed per u)
    nc = bacc.Bacc("TRN2", target_bir_lowering=False, debug=False,
                   num_devices=N_CORES)

    idt = fp8 if MM_FP8 else bf16
    xs = nc.dram_tensor("xs", [nsteps, 64, S], idt, kind="ExternalInput").ap()
    if MM_FP8:
        wconv = nc.dram_tensor("wconv", [128, 2, 6, 2, 128], fp8,
                               kind="ExternalInput").ap()
    else:
        wconv = nc.dram_tensor("wconv", [9, 2, 128, 128], bf16,
                               kind="ExternalInput").ap()
    wfu = nc.dram_tensor("wfu", [64, 64], bf16, kind="ExternalInput").ap()
    gnw = nc.dram_tensor("gnw", [2, 128], fp32, kind="ExternalInput").ap()
    gnb = nc.dram_tensor("gnb", [2, 128], fp32, kind="ExternalInput").ap()
    bnw = nc.dram_tensor("bnw", [64, 1], fp32, kind="ExternalInput").ap()
    bnb = nc.dram_tensor("bnb", [64, 1], fp32, kind="ExternalInput").ap()
    ind = nc.dram_tensor("ind", [128, 2], fp32, kind="ExternalInput").ap()
    indT = nc.dram_tensor("indT", [2, 128], fp32, kind="ExternalInput").ap()
    bind = nc.dram_tensor("bind", [128, 64], fp32, kind="ExternalInput").ap()
    bindT = nc.dram_tensor("bindT", [64, 128], fp32, kind="ExternalInput").ap()
    wsel = nc.dram_tensor("wsel", [128, 256], bf16, kind="ExternalInput").ap()
    ident = nc.dram_tensor("ident", [128, 128], bf16, kind="ExternalInput").ap()
    out = nc.dram_tensor("out", [nlate, 64, S], fp32, kind="ExternalOutput").ap()

    psend = nc.dram_tensor("psend", [nsteps, 64, S], bf16)
    pgath_a = nc.dram_tensor("pgath_a", [2, nsteps // 4, 64, S], bf16)
    pgath_b = nc.dram_tensor("pgath_b", [2, nsteps // 4, 64, S], bf16)
    bnps = nc.dram_tensor("bnps", [64, 2], fp32)
    bnpr = nc.dram_tensor("bnpr", [64, 2], fp32, addr_space="Shared")

    with tile.TileContext(nc) as tc:
      with tc.tile_pool(name="const", bufs=1) as cp:
        wsel_sb = cp.tile([128, 256], bf16, tag="wsel")
        nc.sync.dma_start(wsel_sb[:], wsel)
        ident_sb = cp.tile([128, 128], bf16, tag="ident")
        nc.sync.dma_start(ident_sb[:], ident)
        bind_r = cp.tile([128, 64], fp32, tag="bindr")
        nc.sync.dma_start(bind_r[:], bind)
        bindT_r = cp.tile([64, 128], fp32, tag="bindTr")
        nc.sync.dma_start(bindT_r[:], bindT)
        bnw_sb = cp.tile([64, 1], fp32, tag="bnw")
        nc.sync.dma_start(bnw_sb[:], bnw)
        bnb_sb = cp.tile([64, 1], fp32, tag="bnb")
        nc.sync.dma_start(bnb_sb[:], bnb)

        for rep in range(reps):
          with (
            tc.tile_pool(name=f"persist{rep}", bufs=1) as pp,
            tc.tile_pool(name=f"work{rep}", bufs=2) as wp,
            tc.tile_pool(name=f"pg{rep}", bufs=3, space="PSUM") as pgpool,
            tc.tile_pool(name=f"sm{rep}", bufs=2, space="PSUM") as smpool,
          ):
            # ---- one-time prologue ----
            if MM_FP8:
                wr = pp.tile([128, 2 * 6 * 2 * 128], fp8, tag="wr")
                nc.sync.dma_start(
                    wr[:].rearrange("k (h p two m) -> k h p two m", h=2, p=6, two=2),
                    wconv)
                wrv = wr[:].rearrange("k (h p two m) -> k h p two m", h=2, p=6, two=2)
            else:
                wr = pp.tile([128, 18 * 128], bf16, tag="wr")
                nc.sync.dma_start(
                    wr[:].rearrange("k (t h m) -> k t h m", t=9, h=2),
                    wconv.rearrange("t h k m -> k t h m"),
                )
            wfu_sb = pp.tile([128, 64], bf16, tag="wfusb")
            nc.sync.dma_start(wfu_sb[0:64 if MM_FP8 else slice(64, 128).start:
                                     64 if not MM_FP8 else 64, :]
                              if False else
                              (wfu_sb[0:64, :] if MM_FP8 else wfu_sb[64:128, :]),
                              wfu)

            ind_r = pp.tile([128, 2], fp32, tag="indr")
            nc.sync.dma_start(ind_r[:], ind)
            indT_r = pp.tile([2, 128], fp32, tag="indTr")
            nc.sync.dma_start(indT_r[:], indT)
            gnw_sb = pp.tile([128, 2], fp32, tag="gnw")
            nc.sync.dma_start(gnw_sb[:], gnw.rearrange("h p -> p h"))
            gnb_sb = pp.tile([128, 2], fp32, tag="gnb")
            nc.sync.dma_start(gnb_sb[:], gnb.rearrange("h p -> p h"))

            # persistent state
            inp0 = pp.tile([128, ROWS, PITCH], idt, tag="inp0")
            inp1 = pp.tile([128, ROWS, PITCH], idt, tag="inp1")
            nc.vector.memset(inp0[:], 0.0)
            nc.vector.memset(inp1[:], 0.0)
            inps = [inp0, inp1]
            if_sb = pp.tile([128, S], bf16, tag="ifsb")   # i(0:64), f(64:128)
            og_sb = pp.tile([128, S], bf16, tag="ogsb")   # g(0:64), o(64:128)
            # c / f*c staging live on partitions 64:128 so every TensorTensor
            # pairing them with f,o (if_sb/og_sb rows 64:128) has equal input
            # base partitions (a walrus requirement for 2-input SB ops)
            state_t = pp.tile([128, S], bf16, tag="state")
            state = state_t[64:128, :]
            nc.vector.memset(state, 0.0)
            fc_t = pp.tile([128, S], bf16, tag="fcall")
            fc_all = fc_t[64:128, :]
            if MM_FP8:
                hb = pp.tile([64, 64, 64], bf16, tag="hb")

            # x(0) load straight into the padded input tile
            nc.sync.dma_start(inp0[0:64, 1:65, 1:65],
                              xs[0].rearrange("p (a b) -> p a b", a=64))

            for t in range(nsteps):
                cur = inps[t % 2]
                nxt = inps[(t + 1) % 2]
                if t < nsteps - 1:
                    nc.sync.dma_start(nxt[0:64, 1:65, 1:65],
                                      xs[t + 1].rearrange("p (a b) -> p a b", a=64))
                svs, bvs = [], []
                for half in range(2):
                    raw = if_sb if half == 0 else og_sb
                    st_t = wp.tile([128, 4, 6], fp32, tag=f"st{half}")
                    for qc in range(4):
                        pg = pgpool.tile([128, 1024], fp32, tag="pg")
                        if MM_FP8:
                            # 6 DoubleRow pairs: ((0,c),(1,c)) and ((2,c),zero)
                            for b in range(2):
                                y0 = qc * 16 + b * 8
                                for p in range(6):
                                    dyA, dxA = (0, p) if p < 3 else (2, p - 3)
                                    base = cur[:, y0 + dyA:y0 + dyA + 8,
                                               dxA:dxA + 64]
                                    rhs = bass.AP(
                                        base.tensor, base.offset,
                                        [list(base.ap[0]), [PITCH, 2],
                                         [PITCH, 8], [1, 64]])
                                    nc.tensor.matmul(
                                        pg[:, b * 512:(b + 1) * 512],
                                        wrv[:, half, p], rhs,
                                        start=(p == 0), stop=(p == 5),
                                        perf_mode=PM.DoubleRow)
                        else:
                            for tap in range(9):
                                dy, dx = tap // 3, tap % 3
                                lhsT = wr[:, (tap * 2 + half) * 128:(tap * 2 + half + 1) * 128]
                                for b in range(2):
                                    y0 = qc * 16 + b * 8
                                    nc.tensor.matmul(
                                        pg[:, b * 512:(b + 1) * 512],
                                        lhsT,
                                        cur[:, y0 + dy:y0 + dy + 8, dx:dx + 64],
                                        start=(tap == 0), stop=(tap == 8),
                                    )
                        nc.vector.bn_stats(st_t[:, qc, :], pg[:, 0:1024:2])
                        ceng = (nc.scalar.copy if (half * 4 + qc) % 8 < 6
                                else nc.vector.tensor_copy)
                        ceng(raw[:, qc * 1024:(qc + 1) * 1024], pg[:])

                    # group-norm stats chain for this half
                    aggr = wp.tile([128, 2], fp32, tag=f"aggr{half}")
                    nc.vector.bn_aggr(aggr[:], st_t[:])
                    s3 = wp.tile([128, 3], fp32, tag=f"s3{half}")
                    nc.vector.tensor_copy(s3[:, 0:2], aggr[:])
                    nc.vector.tensor_mul(s3[:, 2:3], aggr[:, 0:1], aggr[:, 0:1])
                    smg = smpool.tile([2, 3], fp32, tag="sm")
                    nc.tensor.matmul(smg[:], ind_r[:], s3[:], start=True, stop=True)
                    gsb = wp.tile([2, 3], fp32, tag=f"gsb{half}")
                    nc.vector.tensor_scalar_mul(gsb[:], smg[:], 1.0 / 64.0)
                    mu2 = wp.tile([2, 1], fp32, tag=f"mu2{half}")
                    nc.vector.tensor_mul(mu2[:], gsb[:, 0:1], gsb[:, 0:1])
                    varx = wp.tile([2, 1], fp32, tag=f"varx{half}")
                    nc.vector.tensor_add(varx[:], gsb[:, 1:2], gsb[:, 2:3])
                    nc.vector.scalar_tensor_tensor(varx[:], varx[:], EPS, mu2[:],
                                                   Alu.add, Alu.subtract)
                    rstd = wp.tile([2, 1], fp32, tag=f"rstd{half}")
                    emit_rsqrt(nc, wp, varx[:], rstd[:])
                    brhs = wp.tile([2, 2], fp32, tag=f"brhs{half}")
                    nc.vector.tensor_copy(brhs[:, 0:1], rstd[:])
                    nc.vector.tensor_copy(brhs[:, 1:2], gsb[:, 0:1])
                    smb = smpool.tile([128, 2], fp32, tag="sm")
                    nc.tensor.matmul(smb[:], indT_r[:], brhs[:], start=True, stop=True)
                    sv = wp.tile([128, 1], fp32, tag=f"sv{half}")
                    nc.vector.tensor_mul(sv[:], smb[:, 0:1], gnw_sb[:, half:half + 1])
                    tv = wp.tile([128, 1], fp32, tag=f"tv{half}")
                    nc.vector.tensor_mul(tv[:], smb[:, 1:2], sv[:])
                    bv = wp.tile([128, 1], fp32, tag=f"bv{half}")
                    nc.vector.tensor_sub(bv[:], gnb_sb[:, half:half + 1], tv[:])
                    if half == 1:
                        # g is computed as sigma(2z) (tanh identity): double
                        # scale+bias on the g rows so one sigmoid pass covers g,o
                        nc.vector.tensor_scalar_mul(sv[0:64, :], sv[0:64, :], 2.0)
                        nc.vector.tensor_scalar_mul(bv[0:64, :], bv[0:64, :], 2.0)
                    svs.append(sv)
                    bvs.append(bv)
                    if half == 0:
                        for ch in range(NCH):
                            sl = slice(ch * CHUNK, (ch + 1) * CHUNK)
                            nc.scalar.activation(if_sb[:, sl], if_sb[:, sl],
                                                 Act.Sigmoid, bias=bv[:],
                                                 scale=sv[:])
                        for ch in range(NCH):
                            sl = slice(ch * CHUNK, (ch + 1) * CHUNK)
                            nc.vector.tensor_mul(fc_all[:, sl],
                                                 if_sb[64:128, sl],
                                                 state[:, sl])

                # gates + state update, chunked; h lands in nxt per chunk
                psb = wp.tile([64, S], bf16, tag="psb")
                for ch in range(NCH):
                    sl = slice(ch * CHUNK, (ch + 1) * CHUNK)
                    rows = CHUNK // 64
                    r0 = ch * rows
                    nc.scalar.activation(og_sb[:, sl], og_sb[:, sl], Act.Sigmoid,
                                         bias=bvs[1][:], scale=svs[1][:])
                    # g = tanh(z) = 2*sigma(2z)-1, applied in place
                    ggeng = nc.gpsimd if MM_FP8 else nc.vector
                    ggeng.tensor_scalar(og_sb[0:64, sl], og_sb[0:64, sl],
                                        2.0, -1.0, Alu.mult, Alu.add)
                    ig_t = wp.tile([128, CHUNK], bf16, tag="ig")
                    ig = ig_t[64:128, :]
                    nc.vector.tensor_mul(ig, if_sb[0:64, sl], og_sb[0:64, sl])
                    nc.vector.tensor_add(state[:, sl], fc_all[:, sl], ig)
                    th_t = wp.tile([128, CHUNK], bf16, tag="th")
                    th = th_t[64:128, :]
                    nc.scalar.activation(th, state[:, sl], Act.Tanh)
                    if MM_FP8:
                        nc.gpsimd.tensor_mul(
                            nxt[64:128, 1 + r0:1 + r0 + rows, 1:65],
                            og_sb[64:128, sl].rearrange("p (a b) -> p a b", a=rows),
                            th.rearrange("p (a b) -> p a b", a=rows),
                        )
                        nc.vector.tensor_mul(
                            hb[:, r0:r0 + rows, :],
                            og_sb[64:128, sl].rearrange("p (a b) -> p a b", a=rows),
                            th.rearrange("p (a b) -> p a b", a=rows),
                        )
                        fu_lhs, fu_src, off = wfu_sb[0:64, :], hb, 0
                    else:
                        nc.gpsimd.tensor_mul(
                            nxt[64:128, 1 + r0:1 + r0 + rows, 1:65],
                            og_sb[64:128, sl].rearrange("p (a b) -> p a b", a=rows),
                            th.rearrange("p (a b) -> p a b", a=rows),
                        )
                        fu_lhs, fu_src, off = wfu_sb[64:128, :], nxt, 1
                    # fusion partials for the 2 8-row slices of this chunk
                    for kk in (2 * ch, 2 * ch + 1):
                        pf = smpool.tile([64, 512], fp32, tag="sm")
                        if MM_FP8:
                            rhsf = fu_src[:, kk * 8:kk * 8 + 8, :]
                        else:
                            rhsf = fu_src[64:128, 1 + kk * 8:1 + kk * 8 + 8, 1:65]
                        nc.tensor.matmul(pf[:], fu_lhs, rhsf,
                                         start=True, stop=True)
                        peng = nc.scalar.copy if kk % 2 == 0 else nc.vector.tensor_copy
                        peng(psb[:, kk * 512:(kk + 1) * 512], pf[:])
                nc.sync.dma_start(psend[t], psb[:])

                if with_tail and t == nsteps // 4 - 1:
                    nc.gpsimd.collective_compute(
                        "AllGather", Alu.bypass,
                        replica_groups=[[0, 1], [2, 3], [4, 5], [6, 7]],
                        ins=[psend[0:nsteps // 4]],
                        outs=[pgath_a[:]],
                    )
                if with_tail and t == nsteps // 2 - 1:
                    nc.gpsimd.collective_compute(
                        "AllGather", Alu.bypass,
                        replica_groups=[[0, 1], [2, 3], [4, 5], [6, 7]],
                        ins=[psend[nsteps // 4:nsteps // 2]],
                        outs=[pgath_b[:]],
                    )

          # ---- fusion/BN tail (time-split: local j = nlate..nsteps-1) ----
          if not with_tail:
              continue
          with (
            tc.tile_pool(name=f"tailp{rep}", bufs=1) as tp,
            tc.tile_pool(name=f"tailw{rep}", bufs=2) as tw,
            tc.tile_pool(name=f"tsm{rep}", bufs=2, space="PSUM") as tsm,
          ):
            Fsb = tp.tile([128, nu, S], bf16, tag="Fsb")
            st2 = tp.tile([128, 8 * nu, 6], fp32, tag="st2")
            for u in range(nu):
                j0 = nlate + 2 * u
                j1 = j0 + 1
                def pga(rank, i):
                    q = nsteps // 4
                    return pgath_a[rank, i] if i < q else pgath_b[rank, i - q]
                i0 = nsteps - 1 - j0
                i1 = nsteps - 1 - j1
                L0 = tw.tile([128, S], bf16, tag="L0")
                nc.sync.dma_start(L0[0:64, :], pga(0, i0))
                nc.sync.dma_start(L0[64:128, :], pga(1, i0))
                L1 = tw.tile([128, S], bf16, tag="L1")
                nc.sync.dma_start(L1[0:64, :], pga(0, i1))
                nc.sync.dma_start(L1[64:128, :], pga(1, i1))
                P2 = tw.tile([128, S], bf16, tag="P2")
                nc.sync.dma_start(P2[0:64, :], psend[j0])
                nc.sync.dma_start(P2[64:128, :], psend[j1])
                for q in range(8):
                    fs = slice(q * 512, (q + 1) * 512)
                    F2 = tsm.tile([128, 512], fp32, tag="tsm")
                    nc.tensor.matmul(F2[:], wsel_sb[:, 0:128], L0[:, fs],
                                     start=True, stop=False)
                    nc.tensor.matmul(F2[:], wsel_sb[:, 128:256], L1[:, fs],
                                     start=False, stop=False)
                    nc.tensor.matmul(F2[:], ident_sb[:], P2[:, fs],
                                     start=False, stop=True)
                    nc.vector.bn_stats(st2[:, u * 8 + q, :], F2[:])
                    eng = (nc.scalar.copy if q % 2 == 0 else nc.vector.tensor_copy)
                    eng(Fsb[:, u, fs], F2[:])

            aggr = tw.tile([128, 2], fp32, tag="taggr")
            nc.vector.bn_aggr(aggr[:], st2[:])
            s2 = tw.tile([128, 2], fp32, tag="ts2")
            nc.vector.tensor_copy(s2[:, 0:1], aggr[:, 0:1])
            t128 = tw.tile([128, 1], fp32, tag="t128")
            nc.vector.tensor_mul(t128[:], aggr[:, 0:1], aggr[:, 0:1])
            nc.vector.tensor_add(s2[:, 1:2], aggr[:, 1:2], t128[:])
            smg = tsm.tile([64, 2], fp32, tag="tsm")
            nc.tensor.matmul(smg[:], bind_r[:], s2[:], start=True, stop=True)
            bsb = tw.tile([64, 2], fp32, tag="bsb")
            nc.scalar.copy(bsb[:], smg[:])
            nc.sync.dma_start(bnps[:], bsb[:])
            nc.gpsimd.collective_compute(
                "AllReduce", Alu.add,
                replica_groups=[CORE_IDS],
                ins=[bnps[:]], outs=[bnpr[:]],
            )
            s16 = tw.tile([64, 2], fp32, tag="s16")
            nc.sync.dma_start(s16[:], bnpr[:])
            mE = tw.tile([64, 2], fp32, tag="mE")
            nc.vector.tensor_scalar_mul(mE[:], s16[:], 1.0 / 16.0)
            mu2 = tw.tile([64, 1], fp32, tag="tmu2")
            nc.vector.tensor_mul(mu2[:], mE[:, 0:1], mE[:, 0:1])
            varx = tw.tile([64, 1], fp32, tag="tvarx")
            nc.vector.scalar_tensor_tensor(varx[:], mE[:, 1:2], EPS, mu2[:],
                                           Alu.add, Alu.subtract)
            rstd = tw.tile([64, 1], fp32, tag="trstd")
            emit_rsqrt(nc, tw, varx[:], rstd[:])
            brhs = tw.tile([64, 2], fp32, tag="tbrhs")
            nc.vector.tensor_mul(brhs[:, 0:1], bnw_sb[:], rstd[:])
            tv = tw.tile([64, 1], fp32, tag="ttv")
            nc.vector.tensor_mul(tv[:], mE[:, 0:1], brhs[:, 0:1])
            nc.vector.tensor_sub(brhs[:, 1:2], bnb_sb[:], tv[:])
            smb = tsm.tile([128, 2], fp32, tag="tsm")
            nc.tensor.matmul(smb[:], bindT_r[:], brhs[:], start=True, stop=True)
            svec = tw.tile([128, 1], fp32, tag="tsvec")
            nc.vector.tensor_copy(svec[:], smb[:, 0:1])
            bvec = tw.tile([128, 1], fp32, tag="tbvec")
            nc.vector.tensor_copy(bvec[:], smb[:, 1:2])

            for u in range(nu):
                R = tw.tile([128, S], fp32, tag="R")
                for cc in range(2):
                    cols = slice(cc * 2048, (cc + 1) * 2048)
                    nc.scalar.activation(R[:, cols], Fsb[:, u, cols], Act.Relu,
                                         bias=bvec[:], scale=svec[:])
                nc.sync.dma_start(out[2 * u], R[0:64, :])
                nc.sync.dma_start(out[2 * u + 1], R[64:128, :])

    nc.compile()
    return nc


def make_in_maps(x, Wf, gnf_w, gnf_b, Wb, gnb_w, gnb_b, Wfu, bn_w, bn_b,
                 nsteps=T):
    perm = np.concatenate([np.arange(0, 128), np.arange(192, 256),
                           np.arange(128, 192)])
    ind_m = np.zeros((128, 2), np.float32)
    ind_m[0:64, 0] = 1.0
    ind_m[64:128, 1] = 1.0
    indT_m = np.ascontiguousarray(ind_m.T)
    bind_m = np.zeros((128, 64), np.float32)
    for c in range(64):
        bind_m[c, c] = 1.0
        bind_m[c + 64, c] = 1.0
    bindT_m = np.ascontiguousarray(bind_m.T)
    Wfu2 = np.asarray(Wfu)[:, :, 0, 0]

    def to_bf16(a):
        import jax.numpy as jnp
        return np.asarray(jnp.asarray(a, dtype=jnp.bfloat16))

    def to_fp8(a):
        import ml_dtypes
        return np.asarray(a, np.float32).astype(ml_dtypes.float8_e4m3)

    ident_bf = to_bf16(np.eye(128, dtype=np.float32))

    in_maps = []
    for core in range(N_CORES):
        b = core // 2
        fwd = core % 2 == 0
        xb = np.asarray(x)[b].reshape(-1, 64, S)[:nsteps]
        if not fwd:
            xb = xb[::-1]
        Wd = np.asarray(Wf if fwd else Wb)[perm]
        gw = np.asarray(gnf_w if fwd else gnb_w)[perm]
        gb = np.asarray(gnf_b if fwd else gnb_b)[perm]
        wconv_m = np.empty((9, 2, 128, 128), np.float32)
        for tap in range(9):
            dy, dx = tap // 3, tap % 3
            for half in range(2):
                wconv_m[tap, half] = Wd[half * 128:(half + 1) * 128, :, dy, dx].T
        if MM_FP8:
            # [k, half, pair, 2, m]: pairs ((0,c),(1,c)) c<3, ((2,c), zero)
            wdr_m = np.zeros((128, 2, 6, 2, 128), np.float32)
            for half in range(2):
                for p in range(6):
                    if p < 3:
                        tA, tB = p, p + 3
                        wdr_m[:, half, p, 1] = wconv_m[tB, half] * W8SCALE
                    else:
                        tA = 6 + (p - 3)
                    wdr_m[:, half, p, 0] = wconv_m[tA, half] * W8SCALE
            wconv_in = to_fp8(wdr_m)
        else:
            wconv_in = to_bf16(wconv_m)
        wfu_m = (Wfu2[:, 0:64] if fwd else Wfu2[:, 64:128]).T

        # peer selector: L rows 0:64 = rank0 (fwd) data, 64:128 = rank1 (bwd).
        # Each core picks the OTHER core's rows.
        peer_base = 64 if fwd else 0
        wsel_m = np.zeros((128, 256), np.float32)
        for m in range(64):
            wsel_m[peer_base + m, m] = 1.0          # lhsT_a -> F2 rows 0:64
            wsel_m[peer_base + m, 128 + 64 + m] = 1.0  # lhsT_b -> F2 rows 64:128
        in_maps.append({
            "xs": to_fp8(xb) if MM_FP8 else to_bf16(xb),
            "wconv": wconv_in,
            "wfu": to_bf16(wfu_m),
            "gnw": np.ascontiguousarray(gw.reshape(2, 128)),
            "gnb": np.ascontiguousarray(gb.reshape(2, 128)),
            "bnw": np.asarray(bn_w, np.float32).reshape(64, 1).copy(),
            "bnb": np.asarray(bn_b, np.float32).reshape(64, 1).copy(),
            "ind": ind_m,
            "indT": indT_m,
            "bind": bind_m,
            "bindT": bindT_m,
            "wsel": to_bf16(wsel_m),
            "ident": ident_bf,
        })
    return in_maps


_cached_nc = None


def kernel(x, Wf, gnf_w, gnf_b, Wb, gnb_w, gnb_b, Wfu, bn_w, bn_b):
    global _cached_nc
    if _cached_nc is None:
        _cached_nc = build_program(T)
    nc = _cached_nc
    in_maps = make_in_maps(x, Wf, gnf_w, gnf_b, Wb, gnb_w, gnb_b, Wfu,
                           bn_w, bn_b)
    res = run_bass_kernel_spmd(nc, in_maps, CORE_IDS)
    B = 4
    outf = np.empty((B, T, HID, 64, 64), np.float32)
    for b in range(B):
        fo = res.results[2 * b]["out"].reshape(T // 2, HID, 64, 64)
        bo = res.results[2 * b + 1]["out"].reshape(T // 2, HID, 64, 64)
        for s in range(T // 2):
            outf[b, T // 2 + s] = fo[s]      # fwd core: local j=8+s -> true 8+s
            outf[b, T // 2 - 1 - s] = bo[s]  # bwd core: local j=8+s -> true 7-s
    return np.ascontiguousarray(outf)


# revision 18
# speedup vs baseline: 1.1310x; 1.0016x over previous
"""Bidirectional ConvLSTM Trainium2 kernel (8-core SPMD), v2.

Sharding: 8 sequences = 4 batches x 2 directions; core 2b = forward for
batch b, core 2b+1 = backward (host feeds time-reversed x and that
direction's weights). bf16 datapath (conv matmuls, gates, c/h state).

Fusion/BN tail is time-split: each core finalizes its LOCAL steps
j = T/2..T-1 (true t = j on fwd cores, 15-j on bwd cores). Each core's
EARLY-half fusion partials (local t < T/2) are AllGather'd pairwise
mid-loop (hidden under compute); the tail combines local psend[j] with
the peer's pgath[T-1-j] via PE matmuls using host-provided selector
weights (wsel) that encode core parity. BatchNorm stats via an 8-core
AllReduce; host reassembles the time axis from both cores of each pair.
"""

import os

import numpy as np
import concourse.bass as bass
import concourse.bacc as bacc
import concourse.mybir as mybir
import concourse.tile as tile
from concourse.bass_utils import run_bass_kernel_spmd

fp32 = mybir.dt.float32
bf16 = mybir.dt.bfloat16
fp8 = mybir.dt.float8e4
PM = mybir.MatmulPerfMode
MM_FP8 = os.environ.get("MM_FP8", "0") == "1"  # fp8e4m3 DoubleRow conv
W8SCALE = 64.0     # fp8 conv-weight prescale; cancels exactly in GroupNorm
PITCH = 80         # input-tile row pitch (16B-aligned for the DR pair stride)
ROWS = 67          # 66 + 1 guard row for the dy+1 pair read of the last taps
i32 = mybir.dt.int32
Alu = mybir.AluOpType
Act = mybir.ActivationFunctionType

T = 16
HID = 64
S = 4096  # 64*64 spatial
EPS = 1e-5
N_CORES = 8
CORE_IDS = list(range(N_CORES))
MAGIC = 0x5F3759DF
CHUNK = 1024  # half-0 pre-pass chunk
NCH = S // CHUNK
CHUNKS = [512, 1024, 1024, 1536]  # gate-phase chunks (graduated)


def emit_rsqrt(nc, pool, x_ap, out_ap, iters=2):
    """out = 1/sqrt(x) via bit-trick seed + Newton, DVE only. x_ap fp32 [P,1]."""
    P = x_ap.shape[0]
    yi = pool.tile([P, 1], i32, tag=f"rsq_i{P}")
    t = pool.tile([P, 1], fp32, tag=f"rsq_t{P}")
    nc.vector.tensor_scalar(yi[:], x_ap.bitcast(i32), 1, None,
                            Alu.logical_shift_right)
    nc.vector.tensor_scalar(yi[:], yi[:], -1, MAGIC, Alu.mult, Alu.add)
    y = yi[:].bitcast(fp32)
    for i in range(iters):
        nc.vector.tensor_mul(t[:], y, y)
        nc.vector.tensor_mul(t[:], t[:], x_ap)
        nc.vector.tensor_scalar(t[:], t[:], -0.5, 1.5, Alu.mult, Alu.add)
        nc.vector.tensor_mul(out_ap if i == iters - 1 else y, y, t[:])


def build_program(nsteps=T, reps=1, with_tail=True):
    assert nsteps % 4 == 0
    nlate = nsteps // 2          # steps this core finalizes (local j >= nlate)
    nu = nlate // 2              # tail iterations (2 steps packed per u)
    nc = bacc.Bacc("TRN2", target_bir_lowering=False, debug=False,
                   num_devices=N_CORES)

    idt = fp8 if MM_FP8 else bf16
    xs = nc.dram_tensor("xs", [nsteps, 64, S], idt, kind="ExternalInput").ap()
    if MM_FP8:
        wconv = nc.dram_tensor("wconv", [128, 2, 6, 2, 128], fp8,
                               kind="ExternalInput").ap()
    else:
        wconv = nc.dram_tensor("wconv", [9, 2, 128, 128], bf16,
                               kind="ExternalInput").ap()
    wfu = nc.dram_tensor("wfu", [64, 64], bf16, kind="ExternalInput").ap()
    gnw = nc.dram_tensor("gnw", [2, 128], fp32, kind="ExternalInput").ap()
    gnb = nc.dram_tensor("gnb", [2, 128], fp32, kind="ExternalInput").ap()
    bnw = nc.dram_tensor("bnw", [64, 1], fp32, kind="ExternalInput").ap()
    bnb = nc.dram_tensor("bnb", [64, 1], fp32, kind="ExternalInput").ap()
    ind = nc.dram_tensor("ind", [128, 2], fp32, kind="ExternalInput").ap()
    indT = nc.dram_tensor("indT", [2, 128], fp32, kind="ExternalInput").ap()
    bind = nc.dram_tensor("bind", [128, 64], fp32, kind="ExternalInput").ap()
    bindT = nc.dram_tensor("bindT", [64, 128], fp32, kind="ExternalInput").ap()
    wsel = nc.dram_tensor("wsel", [128, 256], bf16, kind="ExternalInput").ap()
    ident = nc.dram_tensor("ident", [128, 128], bf16, kind="ExternalInput").ap()
    out = nc.dram_tensor("out", [nlate, 64, S], fp32, kind="ExternalOutput").ap()

    psend = nc.dram_tensor("psend", [nsteps, 64, S], bf16)
    pgath_a = nc.dram_tensor("pgath_a", [2, nsteps // 4, 64, S], bf16)
    pgath_b = nc.dram_tensor("pgath_b", [2, nsteps // 4, 64, S], bf16)
    bnps = nc.dram_tensor("bnps", [64, 2], fp32)
    bnpr = nc.dram_tensor("bnpr", [64, 2], fp32, addr_space="Shared")

    with tile.TileContext(nc) as tc:
      with tc.tile_pool(name="const", bufs=1) as cp:
        wsel_sb = cp.tile([128, 256], bf16, tag="wsel")
        nc.sync.dma_start(wsel_sb[:], wsel)
        ident_sb = cp.tile([128, 128], bf16, tag="ident")
        nc.sync.dma_start(ident_sb[:], ident)
        bind_r = cp.tile([128, 64], fp32, tag="bindr")
        nc.sync.dma_start(bind_r[:], bind)
        bindT_r = cp.tile([64, 128], fp32, tag="bindTr")
        nc.sync.dma_start(bindT_r[:], bindT)
        bnw_sb = cp.tile([64, 1], fp32, tag="bnw")
        nc.sync.dma_start(bnw_sb[:], bnw)
        bnb_sb = cp.tile([64, 1], fp32, tag="bnb")
        nc.sync.dma_start(bnb_sb[:], bnb)

        for rep in range(reps):
          with (
            tc.tile_pool(name=f"persist{rep}", bufs=1) as pp,
            tc.tile_pool(name=f"work{rep}", bufs=2) as wp,
            tc.tile_pool(name=f"pg{rep}", bufs=3, space="PSUM") as pgpool,
            tc.tile_pool(name=f"sm{rep}", bufs=2, space="PSUM") as smpool,
          ):
            # ---- one-time prologue ----
            if MM_FP8:
                wr = pp.tile([128, 2 * 6 * 2 * 128], fp8, tag="wr")
                nc.sync.dma_start(
                    wr[:].rearrange("k (h p two m) -> k h p two m", h=2, p=6, two=2),
                    wconv)
                wrv = wr[:].rearrange("k (h p two m) -> k h p two m", h=2, p=6, two=2)
            else:
                wr = pp.tile([128, 18 * 128], bf16, tag="wr")
                nc.sync.dma_start(
                    wr[:].rearrange("k (t h m) -> k t h m", t=9, h=2),
                    wconv.rearrange("t h k m -> k t h m"),
                )
            wfu_sb = pp.tile([128, 64], bf16, tag="wfusb")
            nc.sync.dma_start(wfu_sb[0:64 if MM_FP8 else slice(64, 128).start:
                                     64 if not MM_FP8 else 64, :]
                              if False else
                              (wfu_sb[0:64, :] if MM_FP8 else wfu_sb[64:128, :]),
                              wfu)

            ind_r = pp.tile([128, 2], fp32, tag="indr")
            nc.sync.dma_start(ind_r[:], ind)
            indT_r = pp.tile([2, 128], fp32, tag="indTr")
            nc.sync.dma_start(indT_r[:], indT)
            gnw_sb = pp.tile([128, 2], fp32, tag="gnw")
            nc.sync.dma_start(gnw_sb[:], gnw.rearrange("h p -> p h"))
            gnb_sb = pp.tile([128, 2], fp32, tag="gnb")
            nc.sync.dma_start(gnb_sb[:], gnb.rearrange("h p -> p h"))

            # persistent state
            inp0 = pp.tile([128, ROWS, PITCH], idt, tag="inp0")
            inp1 = pp.tile([128, ROWS, PITCH], idt, tag="inp1")
            nc.vector.memset(inp0[:], 0.0)
            nc.vector.memset(inp1[:], 0.0)
            inps = [inp0, inp1]
            if_sb = pp.tile([128, S], bf16, tag="ifsb")   # i(0:64), f(64:128)
            og_sb = pp.tile([128, S], bf16, tag="ogsb")   # g(0:64), o(64:128)
            # c / f*c staging live on partitions 64:128 so every TensorTensor
            # pairing them with f,o (if_sb/og_sb rows 64:128) has equal input
            # base partitions (a walrus requirement for 2-input SB ops)
            state_t = pp.tile([128, S], bf16, tag="state")
            state = state_t[64:128, :]
            nc.vector.memset(state, 0.0)
            fc_t = pp.tile([128, S], bf16, tag="fcall")
            fc_all = fc_t[64:128, :]
            if MM_FP8:
                hb = pp.tile([64, 64, 64], bf16, tag="hb")

            # x(0) load straight into the padded input tile
            nc.sync.dma_start(inp0[0:64, 1:65, 1:65],
                              xs[0].rearrange("p (a b) -> p a b", a=64))

            for t in range(nsteps):
                cur = inps[t % 2]
                nxt = inps[(t + 1) % 2]
                if t < nsteps - 1:
                    nc.sync.dma_start(nxt[0:64, 1:65, 1:65],
                                      xs[t + 1].rearrange("p (a b) -> p a b", a=64))
                svs, bvs = [], []
                for half in range(2):
                    raw = if_sb if half == 0 else og_sb
                    st_t = wp.tile([128, 4, 6], fp32, tag=f"st{half}")
                    for qc in range(4):
                        pg = pgpool.tile([128, 1024], fp32, tag="pg")
                        if MM_FP8:
                            # 6 DoubleRow pairs: ((0,c),(1,c)) and ((2,c),zero)
                            for b in range(2):
                                y0 = qc * 16 + b * 8
                                for p in range(6):
                                    dyA, dxA = (0, p) if p < 3 else (2, p - 3)
                                    base = cur[:, y0 + dyA:y0 + dyA + 8,
                                               dxA:dxA + 64]
                                    rhs = bass.AP(
                                        base.tensor, base.offset,
                                        [list(base.ap[0]), [PITCH, 2],
                                         [PITCH, 8], [1, 64]])
                                    nc.tensor.matmul(
                                        pg[:, b * 512:(b + 1) * 512],
                                        wrv[:, half, p], rhs,
                                        start=(p == 0), stop=(p == 5),
                                        perf_mode=PM.DoubleRow)
                        else:
                            for tap in range(9):
                                dy, dx = tap // 3, tap % 3
                                lhsT = wr[:, (tap * 2 + half) * 128:(tap * 2 + half + 1) * 128]
                                for b in range(2):
                                    y0 = qc * 16 + b * 8
                                    nc.tensor.matmul(
                                        pg[:, b * 512:(b + 1) * 512],
                                        lhsT,
                                        cur[:, y0 + dy:y0 + dy + 8, dx:dx + 64],
                                        start=(tap == 0), stop=(tap == 8),
                                    )
                        nc.vector.bn_stats(st_t[:, qc, :], pg[:, 0:1024:2])
                        ceng = (nc.scalar.copy if (not MM_FP8 or
                                (half * 4 + qc) % 8 < 6)
                                else nc.vector.tensor_copy)
                        ceng(raw[:, qc * 1024:(qc + 1) * 1024], pg[:])

                    # group-norm stats chain for this half
                    aggr = wp.tile([128, 2], fp32, tag=f"aggr{half}")
                    nc.vector.bn_aggr(aggr[:], st_t[:])
                    s3 = wp.tile([128, 3], fp32, tag=f"s3{half}")
                    nc.vector.tensor_copy(s3[:, 0:2], aggr[:])
                    nc.vector.tensor_mul(s3[:, 2:3], aggr[:, 0:1], aggr[:, 0:1])
                    smg = smpool.tile([2, 3], fp32, tag="sm")
                    nc.tensor.matmul(smg[:], ind_r[:], s3[:], start=True, stop=True)
                    gsb = wp.tile([2, 3], fp32, tag=f"gsb{half}")
                    nc.vector.tensor_scalar_mul(gsb[:], smg[:], 1.0 / 64.0)
                    mu2 = wp.tile([2, 1], fp32, tag=f"mu2{half}")
                    nc.vector.tensor_mul(mu2[:], gsb[:, 0:1], gsb[:, 0:1])
                    varx = wp.tile([2, 1], fp32, tag=f"varx{half}")
                    nc.vector.tensor_add(varx[:], gsb[:, 1:2], gsb[:, 2:3])
                    nc.vector.scalar_tensor_tensor(varx[:], varx[:], EPS, mu2[:],
                                                   Alu.add, Alu.subtract)
                    rstd = wp.tile([2, 1], fp32, tag=f"rstd{half}")
                    emit_rsqrt(nc, wp, varx[:], rstd[:])
                    brhs = wp.tile([2, 2], fp32, tag=f"brhs{half}")
                    nc.vector.tensor_copy(brhs[:, 0:1], rstd[:])
                    nc.vector.tensor_copy(brhs[:, 1:2], gsb[:, 0:1])
                    smb = smpool.tile([128, 2], fp32, tag="sm")
                    nc.tensor.matmul(smb[:], indT_r[:], brhs[:], start=True, stop=True)
                    sv = wp.tile([128, 1], fp32, tag=f"sv{half}")
                    nc.vector.tensor_mul(sv[:], smb[:, 0:1], gnw_sb[:, half:half + 1])
                    tv = wp.tile([128, 1], fp32, tag=f"tv{half}")
                    nc.vector.tensor_mul(tv[:], smb[:, 1:2], sv[:])
                    bv = wp.tile([128, 1], fp32, tag=f"bv{half}")
                    nc.vector.tensor_sub(bv[:], gnb_sb[:, half:half + 1], tv[:])
                    if half == 1:
                        # g is computed as sigma(2z) (tanh identity): double
                        # scale+bias on the g rows so one sigmoid pass covers g,o
                        nc.vector.tensor_scalar_mul(sv[0:64, :], sv[0:64, :], 2.0)
                        nc.vector.tensor_scalar_mul(bv[0:64, :], bv[0:64, :], 2.0)
                    svs.append(sv)
                    bvs.append(bv)
                    if half == 0:
                        for ch in range(NCH):
                            sl = slice(ch * CHUNK, (ch + 1) * CHUNK)
                            nc.scalar.activation(if_sb[:, sl], if_sb[:, sl],
                                                 Act.Sigmoid, bias=bv[:],
                                                 scale=sv[:])
                        for ch in range(NCH):
                            sl = slice(ch * CHUNK, (ch + 1) * CHUNK)
                            nc.vector.tensor_mul(fc_all[:, sl],
                                                 if_sb[64:128, sl],
                                                 state[:, sl])

                # gates + state update in graduated chunks (small first so
                # the next step's qc0 conv unblocks early). ACT issue order is
                # hand-interleaved: sig-go(ch+1) goes before tanh(ch) so the
                # strict-FIFO ACT queue never idles waiting on the DVE chain.
                psb = wp.tile([64, S], bf16, tag="psb")
                offs = [0]
                for c in CHUNKS:
                    offs.append(offs[-1] + c)

                def sig_go(ch):
                    sl = slice(offs[ch], offs[ch + 1])
                    nc.scalar.activation(og_sb[:, sl], og_sb[:, sl], Act.Sigmoid,
                                         bias=bvs[1][:], scale=svs[1][:])

                sig_go(0)
                for ch in range(len(CHUNKS)):
                    sl = slice(offs[ch], offs[ch + 1])
                    rows = CHUNKS[ch] // 64
                    r0 = offs[ch] // 64
                    if ch + 1 < len(CHUNKS):
                        sig_go(ch + 1)
                    # g = tanh(z) = 2*sigma(2z)-1, applied in place
                    ggeng = nc.gpsimd if MM_FP8 else nc.vector
                    ggeng.tensor_scalar(og_sb[0:64, sl], og_sb[0:64, sl],
                                        2.0, -1.0, Alu.mult, Alu.add)
                    ig_t = wp.tile([128, 1536], bf16, tag="ig")
                    ig = ig_t[64:128, 0:CHUNKS[ch]]
                    nc.vector.tensor_mul(ig, if_sb[0:64, sl], og_sb[0:64, sl])
                    nc.vector.tensor_add(state[:, sl], fc_all[:, sl], ig)
                    th_t = wp.tile([128, 1536], bf16, tag="th")
                    th = th_t[64:128, 0:CHUNKS[ch]]
                    nc.scalar.activation(th, state[:, sl], Act.Tanh)
                    heng = nc.vector if ch == 0 else nc.gpsimd
                    if MM_FP8:
                        heng.tensor_mul(
                            nxt[64:128, 1 + r0:1 + r0 + rows, 1:65],
                            og_sb[64:128, sl].rearrange("p (a b) -> p a b", a=rows),
                            th.rearrange("p (a b) -> p a b", a=rows),
                        )
                        nc.vector.tensor_mul(
                            hb[:, r0:r0 + rows, :],
                            og_sb[64:128, sl].rearrange("p (a b) -> p a b", a=rows),
                            th.rearrange("p (a b) -> p a b", a=rows),
                        )
                        fu_lhs, fu_src = wfu_sb[0:64, :], hb
                    else:
                        heng.tensor_mul(
                            nxt[64:128, 1 + r0:1 + r0 + rows, 1:65],
                            og_sb[64:128, sl].rearrange("p (a b) -> p a b", a=rows),
                            th.rearrange("p (a b) -> p a b", a=rows),
                        )
                        fu_lhs, fu_src = wfu_sb[64:128, :], nxt
                    # fusion partials for the 8-row slices of this chunk
                    for kk in range(offs[ch] // 512, offs[ch + 1] // 512):
                        pf = smpool.tile([64, 512], fp32, tag="sm")
                        if MM_FP8:
                            rhsf = fu_src[:, kk * 8:kk * 8 + 8, :]
                        else:
                            rhsf = fu_src[64:128, 1 + kk * 8:1 + kk * 8 + 8, 1:65]
                        nc.tensor.matmul(pf[:], fu_lhs, rhsf,
                                         start=True, stop=True)
                        peng = nc.scalar.copy if kk % 2 == 0 else nc.vector.tensor_copy
                        peng(psb[:, kk * 512:(kk + 1) * 512], pf[:])
                nc.sync.dma_start(psend[t], psb[:])

                if with_tail and t == nsteps // 4 - 1:
                    nc.gpsimd.collective_compute(
                        "AllGather", Alu.bypass,
                        replica_groups=[[0, 1], [2, 3], [4, 5], [6, 7]],
                        ins=[psend[0:nsteps // 4]],
                        outs=[pgath_a[:]],
                    )
                if with_tail and t == nsteps // 2 - 1:
                    nc.gpsimd.collective_compute(
                        "AllGather", Alu.bypass,
                        replica_groups=[[0, 1], [2, 3], [4, 5], [6, 7]],
                        ins=[psend[nsteps // 4:nsteps // 2]],
                        outs=[pgath_b[:]],
                    )

          # ---- fusion/BN tail (time-split: local j = nlate..nsteps-1) ----
          if not with_tail:
              continue
          with (
            tc.tile_pool(name=f"tailp{rep}", bufs=1) as tp,
            tc.tile_pool(name=f"tailw{rep}", bufs=2) as tw,
            tc.tile_pool(name=f"tsm{rep}", bufs=2, space="PSUM") as tsm,
          ):
            Fsb = tp.tile([128, nu, S], bf16, tag="Fsb")
            st2 = tp.tile([128, 8 * nu, 6], fp32, tag="st2")
            for u in range(nu):
                j0 = nlate + 2 * u
                j1 = j0 + 1
                def pga(rank, i):
                    q = nsteps // 4
                    return pgath_a[rank, i] if i < q else pgath_b[rank, i - q]
                i0 = nsteps - 1 - j0
                i1 = nsteps - 1 - j1
                L0 = tw.tile([128, S], bf16, tag="L0")
                nc.sync.dma_start(L0[0:64, :], pga(0, i0))
                nc.sync.dma_start(L0[64:128, :], pga(1, i0))
                L1 = tw.tile([128, S], bf16, tag="L1")
                nc.sync.dma_start(L1[0:64, :], pga(0, i1))
                nc.sync.dma_start(L1[64:128, :], pga(1, i1))
                P2 = tw.tile([128, S], bf16, tag="P2")
                nc.sync.dma_start(P2[0:64, :], psend[j0])
                nc.sync.dma_start(P2[64:128, :], psend[j1])
                for q in range(8):
                    fs = slice(q * 512, (q + 1) * 512)
                    F2 = tsm.tile([128, 512], fp32, tag="tsm")
                    nc.tensor.matmul(F2[:], wsel_sb[:, 0:128], L0[:, fs],
                                     start=True, stop=False)
                    nc.tensor.matmul(F2[:], wsel_sb[:, 128:256], L1[:, fs],
                                     start=False, stop=False)
                    nc.tensor.matmul(F2[:], ident_sb[:], P2[:, fs],
                                     start=False, stop=True)
                    nc.vector.bn_stats(st2[:, u * 8 + q, :], F2[:])
                    eng = (nc.scalar.copy if q % 2 == 0 else nc.vector.tensor_copy)
                    eng(Fsb[:, u, fs], F2[:])

            aggr = tw.tile([128, 2], fp32, tag="taggr")
            nc.vector.bn_aggr(aggr[:], st2[:])
            s2 = tw.tile([128, 2], fp32, tag="ts2")
            nc.vector.tensor_copy(s2[:, 0:1], aggr[:, 0:1])
            t128 = tw.tile([128, 1], fp32, tag="t128")
            nc.vector.tensor_mul(t128[:], aggr[:, 0:1], aggr[:, 0:1])
            nc.vector.tensor_add(s2[:, 1:2], aggr[:, 1:2], t128[:])
            smg = tsm.tile([64, 2], fp32, tag="tsm")
            nc.tensor.matmul(smg[:], bind_r[:], s2[:], start=True, stop=True)
            bsb = tw.tile([64, 2], fp32, tag="bsb")
            nc.scalar.copy(bsb[:], smg[:])
            nc.sync.dma_start(bnps[:], bsb[:])
            nc.gpsimd.collective_compute(
                "AllReduce", Alu.add,
                replica_groups=[CORE_IDS],
                ins=[bnps[:]], outs=[bnpr[:]],
            )
            s16 = tw.tile([64, 2], fp32, tag="s16")
            nc.sync.dma_start(s16[:], bnpr[:])
            mE = tw.tile([64, 2], fp32, tag="mE")
            nc.vector.tensor_scalar_mul(mE[:], s16[:], 1.0 / 16.0)
            mu2 = tw.tile([64, 1], fp32, tag="tmu2")
            nc.vector.tensor_mul(mu2[:], mE[:, 0:1], mE[:, 0:1])
            varx = tw.tile([64, 1], fp32, tag="tvarx")
            nc.vector.scalar_tensor_tensor(varx[:], mE[:, 1:2], EPS, mu2[:],
                                           Alu.add, Alu.subtract)
            rstd = tw.tile([64, 1], fp32, tag="trstd")
            emit_rsqrt(nc, tw, varx[:], rstd[:])
            brhs = tw.tile([64, 2], fp32, tag="tbrhs")
            nc.vector.tensor_mul(brhs[:, 0:1], bnw_sb[:], rstd[:])
            tv = tw.tile([64, 1], fp32, tag="ttv")
            nc.vector.tensor_mul(tv[:], mE[:, 0:1], brhs[:, 0:1])
            nc.vector.tensor_sub(brhs[:, 1:2], bnb_sb[:], tv[:])
            smb = tsm.tile([128, 2], fp32, tag="tsm")
            nc.tensor.matmul(smb[:], bindT_r[:], brhs[:], start=True, stop=True)
            svec = tw.tile([128, 1], fp32, tag="tsvec")
            nc.vector.tensor_copy(svec[:], smb[:, 0:1])
            bvec = tw.tile([128, 1], fp32, tag="tbvec")
            nc.vector.tensor_copy(bvec[:], smb[:, 1:2])

            for u in range(nu):
                R = tw.tile([128, S], fp32, tag="R")
                for cc in range(2):
                    cols = slice(cc * 2048, (cc + 1) * 2048)
                    nc.scalar.activation(R[:, cols], Fsb[:, u, cols], Act.Relu,
                                         bias=bvec[:], scale=svec[:])
                nc.sync.dma_start(out[2 * u], R[0:64, :])
                nc.sync.dma_start(out[2 * u + 1], R[64:128, :])

    nc.compile()
    return nc


def make_in_maps(x, Wf, gnf_w, gnf_b, Wb, gnb_w, gnb_b, Wfu, bn_w, bn_b,
                 nsteps=T):
    perm = np.concatenate([np.arange(0, 128), np.arange(192, 256),
                           np.arange(128, 192)])
    ind_m = np.zeros((128, 2), np.float32)
    ind_m[0:64, 0] = 1.0
    ind_m[64:128, 1] = 1.0
    indT_m = np.ascontiguousarray(ind_m.T)
    bind_m = np.zeros((128, 64), np.float32)
    for c in range(64):
        bind_m[c, c] = 1.0
        bind_m[c + 64, c] = 1.0
    bindT_m = np.ascontiguousarray(bind_m.T)
    Wfu2 = np.asarray(Wfu)[:, :, 0, 0]

    def to_bf16(a):
        import jax.numpy as jnp
        return np.asarray(jnp.asarray(a, dtype=jnp.bfloat16))

    def to_fp8(a):
        import ml_dtypes
        return np.asarray(a, np.float32).astype(ml_dtypes.float8_e4m3)

    ident_bf = to_bf16(np.eye(128, dtype=np.float32))

    in_maps = []
    for core in range(N_CORES):
        b = core // 2
        fwd = core % 2 == 0
        xb = np.asarray(x)[b].reshape(-1, 64, S)[:nsteps]
        if not fwd:
            xb = xb[::-1]
        Wd = np.asarray(Wf if fwd else Wb)[perm]
        gw = np.asarray(gnf_w if fwd else gnb_w)[perm]
        gb = np.asarray(gnf_b if fwd else gnb_b)[perm]
        wconv_m = np.empty((9, 2, 128, 128), np.float32)
        for tap in range(9):
            dy, dx = tap // 3, tap % 3
            for half in range(2):
                wconv_m[tap, half] = Wd[half * 128:(half + 1) * 128, :, dy, dx].T
        if MM_FP8:
            # [k, half, pair, 2, m]: pairs ((0,c),(1,c)) c<3, ((2,c), zero)
            wdr_m = np.zeros((128, 2, 6, 2, 128), np.float32)
            for half in range(2):
                for p in range(6):
                    if p < 3:
                        tA, tB = p, p + 3
                        wdr_m[:, half, p, 1] = wconv_m[tB, half] * W8SCALE
                    else:
                        tA = 6 + (p - 3)
                    wdr_m[:, half, p, 0] = wconv_m[tA, half] * W8SCALE
            wconv_in = to_fp8(wdr_m)
        else:
            wconv_in = to_bf16(wconv_m)
        wfu_m = (Wfu2[:, 0:64] if fwd else Wfu2[:, 64:128]).T

        # peer selector: L rows 0:64 = rank0 (fwd) data, 64:128 = rank1 (bwd).
        # Each core picks the OTHER core's rows.
        peer_base = 64 if fwd else 0
        wsel_m = np.zeros((128, 256), np.float32)
        for m in range(64):
            wsel_m[peer_base + m, m] = 1.0          # lhsT_a -> F2 rows 0:64
            wsel_m[peer_base + m, 128 + 64 + m] = 1.0  # lhsT_b -> F2 rows 64:128
        in_maps.append({
            "xs": to_fp8(xb) if MM_FP8 else to_bf16(xb),
            "wconv": wconv_in,
            "wfu": to_bf16(wfu_m),
            "gnw": np.ascontiguousarray(gw.reshape(2, 128)),
            "gnb": np.ascontiguousarray(gb.reshape(2, 128)),
            "bnw": np.asarray(bn_w, np.float32).reshape(64, 1).copy(),
            "bnb": np.asarray(bn_b, np.float32).reshape(64, 1).copy(),
            "ind": ind_m,
            "indT": indT_m,
            "bind": bind_m,
            "bindT": bindT_m,
            "wsel": to_bf16(wsel_m),
            "ident": ident_bf,
        })
    return in_maps


_cached_nc = None


def kernel(x, Wf, gnf_w, gnf_b, Wb, gnb_w, gnb_b, Wfu, bn_w, bn_b):
    global _cached_nc
    if _cached_nc is None:
        _cached_nc = build_program(T)
    nc = _cached_nc
    in_maps = make_in_maps(x, Wf, gnf_w, gnf_b, Wb, gnb_w, gnb_b, Wfu,
                           bn_w, bn_b)
    res = run_bass_kernel_spmd(nc, in_maps, CORE_IDS)
    B = 4
    outf = np.empty((B, T, HID, 64, 64), np.float32)
    for b in range(B):
        fo = res.results[2 * b]["out"].reshape(T // 2, HID, 64, 64)
        bo = res.results[2 * b + 1]["out"].reshape(T // 2, HID, 64, 64)
        for s in range(T // 2):
            outf[b, T // 2 + s] = fo[s]      # fwd core: local j=8+s -> true 8+s
            outf[b, T // 2 - 1 - s] = bo[s]  # bwd core: local j=8+s -> true 7-s
    return np.ascontiguousarray(outf)
